# revision 30
# baseline (speedup 1.0000x reference)
"""Trainium2 Bass kernel for BottleNeck attention (8 NeuronCores).

Reference computation (jax, fp32):
    qp = q @ Wq.T + bq          [B=8, L=4096, D=1024]
    kp = k @ Wk.T + bk
    vp = v @ Wv.T + bv
    score = qp[:, :256] @ kp.T / sqrt(D)        [B, 256, L]
    attn  = softmax(score, axis=0)              (softmax over the BATCH axis!)
    out   = attn @ vp                           [B, 256, D]

Strategy:
  * Data-parallel over batch: core c owns batch b=c.
  * Algebraic reassociation avoids projecting full-length k/v AND folds the
    two q-side projections into one via host-precomputed products:
        M  = Wq.T @ Wk          [D, D]   (host, f64)
        m0 = bq @ Wk            [D]      (host)
        w_qkb = Wq.T @ bk       [D]      (host)
        s0 = bq . bk            scalar   (host)
        qk_T[d, q]  = sum_e M[e, d] qT[e, q] + m0[d]          (device)
        qkb[q]      = sum_e w_qkb[e] qT[e, q] + s0            (device)
        score_T = k.T.T @ qk_T (+ ones x qkb row)  [L, Q]
        E = exp(score_T / 32)
        denom = AllReduce_batch(E)      (axis-0 softmax denominator)
        attn_T = E / denom              [L, Q]
        av_T = v.T-chunks @ attn_T      [D, Q]
        out = av_T.T @ Wv.T + rowsum(attn) * bv    [Q, D]
  * Host pre-transposes so no transposes on device.
  * Compute dtype bf16 (fp32 PSUM accumulation); AllReduce payload fp16.
"""

import sys
from contextlib import ExitStack

sys.path.insert(0, "/opt/trn_rl_repo")

import numpy as np

import concourse.bass as bass
import concourse.mybir as mybir
import concourse.tile as tile
from concourse import bacc, bass_utils

B = 8
L = 4096
D = 1024
Q = 256  # bottleneck
N_CORES = 8
P = 128
DC = D // P  # 8 d-chunks
EC = D // P  # 8 e-chunks
LC = L // P  # 32 l-chunks
SCALE = 1.0 / 32.0  # 1/sqrt(1024)

COMPUTE = "bf16"
# AllReduce chunk boundaries (in l-chunks of 128): uneven split so the
# first chunk's AllReduce starts earlier and overlaps more of phase C
AR_LCS = [20, 12]
N_AR = len(AR_LCS)

_CDT = {
    "bf16": mybir.dt.bfloat16,
    "fp32r": mybir.dt.float32r,
    "fp32": mybir.dt.float32,
}

_cached = {}


def _np_cdt():
    if COMPUTE == "bf16":
        import ml_dtypes

        return np.dtype(ml_dtypes.bfloat16)
    return np.dtype(np.float32)


def build_kernel():
    CDT = _CDT[COMPUTE]
    F32 = mybir.dt.float32

    nc = bacc.Bacc("TRN2", target_bir_lowering=False, debug=False,
                   num_devices=N_CORES)

    # ---- per-core external inputs (host pre-transposed / pre-cast) ----
    kT = nc.dram_tensor("kT", [D, L], CDT, kind="ExternalInput")       # k[b].T
    v_in = nc.dram_tensor("v_in", [L, D], CDT, kind="ExternalInput")   # v[b]
    qT = nc.dram_tensor("qT", [D, Q], CDT, kind="ExternalInput")       # q[b,:Q].T
    m_in = nc.dram_tensor("m_in", [D, D], CDT, kind="ExternalInput")   # Wq.T@Wk
    wvT = nc.dram_tensor("wvT", [D, D], CDT, kind="ExternalInput")     # Wv.T
    m0_in = nc.dram_tensor("m0_in", [1, D], CDT, kind="ExternalInput")    # bq@Wk
    wqkb_in = nc.dram_tensor("wqkb_in", [P, EC], CDT, kind="ExternalInput")  # (Wq.T@bk).reshape(EC,P).T
    s0_in = nc.dram_tensor("s0_in", [1, 1], CDT, kind="ExternalInput")    # bq.bk
    bv_in = nc.dram_tensor("bv_in", [1, D], CDT, kind="ExternalInput")
    ones_r_in = nc.dram_tensor("ones_r", [1, Q], CDT, kind="ExternalInput")
    ones_c_in = nc.dram_tensor("ones_c", [P, 1], CDT, kind="ExternalInput")
    out_ext = nc.dram_tensor("out", [Q, D], F32, kind="ExternalOutput")

    # DRAM views with the partition-chunk structure we DMA through
    kT_v = kT.rearrange("(c p) l -> p c l", p=P)        # [128, 8, 4096]
    m_v = m_in.rearrange("(c p) d -> p c d", p=P)       # [128, 8, 1024]
    wvT_v = wvT.rearrange("(c p) e -> p c e", p=P)
    qT_v = qT.rearrange("(c p) q -> p c q", p=P)        # [128, 8, 256]
    v_v = v_in.rearrange("(c p) d -> c p d", p=P)       # [32, 128, 1024]
    out_v = out_ext.rearrange("(m p) e -> p m e", p=P)  # [128, 2, 1024]

    with tile.TileContext(nc) as tc, ExitStack() as top:
        consts = top.enter_context(tc.tile_pool(name="consts", bufs=1))
        qstate = top.enter_context(tc.tile_pool(name="qstate", bufs=1))
        dram = top.enter_context(tc.tile_pool(name="dram", bufs=1, space="DRAM"))

        # ---------------- constants ----------------
        ones_row = consts.tile([1, Q], CDT)       # [1, 256] of 1.0
        ones_col = consts.tile([P, 1], CDT)       # [128, 1] of 1.0
        bv_sb = consts.tile([1, D], CDT)

        qkT_sb = qstate.tile([P, DC, Q], CDT)
        qkb_sb = qstate.tile([1, Q], CDT)
        avT_sb = qstate.tile([P, DC, Q], CDT)
        rs_sb = qstate.tile([1, Q], CDT)

        SLAB = 4  # l-chunks per kT slab (512 l positions)
        kslab_ctx = ExitStack()
        kslabs = kslab_ctx.enter_context(tc.tile_pool(name="kslabs", bufs=3))

        # ================ phase B': qk_T directly from q ================
        with (tc.tile_pool(name="bprime", bufs=1) as bp,
              tc.tile_pool(name="bps4", bufs=4, space="PSUM") as bps4,
              tc.tile_pool(name="bpsk", bufs=1, space="PSUM") as bpsk):
            M_sb = bp.tile([P, EC, D], CDT)
            qT_sb = bp.tile([P, EC, Q], CDT)
            wqkb_sb = bp.tile([P, EC], CDT)
            m0_sb = bp.tile([1, D], CDT)
            s0_sb = bp.tile([1, 1], CDT)
            # per-ec chunks so the first matmuls start after ~300KB of DMA
            for ec in range(EC):
                nc.sync.dma_start(out=M_sb[:, ec, :], in_=m_v[:, ec, :])
                nc.sync.dma_start(out=qT_sb[:, ec, :], in_=qT_v[:, ec, :])
            nc.sync.dma_start(out=wqkb_sb, in_=wqkb_in[:, :])
            nc.sync.dma_start(out=m0_sb, in_=m0_in[:, :])
            nc.sync.dma_start(out=s0_sb, in_=s0_in[:, :])
            nc.sync.dma_start(out=ones_row, in_=ones_r_in[:, :])
            nc.sync.dma_start(out=ones_col, in_=ones_c_in[:, :])
            # prefetch the first kT slabs right behind the B' operands
            kT_pre = []
            for sl in range(2):
                kT_t = kslabs.tile([P, DC, SLAB * P], CDT, tag="kT",
                                   name=f"kT_pre{sl}")
                nc.sync.dma_start(
                    out=kT_t, in_=kT_v[:, :, sl * SLAB * P:(sl + 1) * SLAB * P])
                kT_pre.append(kT_t)

            qkb_ps = bpsk.tile([1, Q], F32)
            for half in range(2):
                qk_ps = [bps4.tile([P, Q], F32, tag="qkps",
                                   name=f"qk_ps_{half}_{i}")
                         for i in range(4)]
                for ec in range(EC):
                    for i, dc in enumerate(range(half * 4, half * 4 + 4)):
                        nc.tensor.matmul(
                            qk_ps[i],
                            M_sb[:, ec, dc * P:(dc + 1) * P],
                            qT_sb[:, ec, :],
                            start=(ec == 0), stop=False,
                        )
                    if half == 0:
                        nc.tensor.matmul(
                            qkb_ps, wqkb_sb[:, ec:ec + 1], qT_sb[:, ec, :],
                            start=(ec == 0), stop=False,
                        )
                for i, dc in enumerate(range(half * 4, half * 4 + 4)):
                    nc.tensor.matmul(
                        qk_ps[i], m0_sb[:, dc * P:(dc + 1) * P], ones_row,
                        start=False, stop=True,
                    )
                    nc.scalar.copy(qkT_sb[:, dc, :], qk_ps[i])
            nc.tensor.matmul(qkb_ps, s0_sb, ones_row, start=False, stop=True)
            nc.scalar.copy(qkb_sb, qkb_ps)

        # ================ phase C: score_T -> E -> DRAM ================
        ps4_ctx = ExitStack()
        ps4 = ps4_ctx.enter_context(
            tc.tile_pool(name="ps4", bufs=6, space="PSUM"))
        bigctx = ExitStack()
        bigbuf = bigctx.enter_context(tc.tile_pool(name="bigbuf", bufs=1))
        wvp = bigctx.enter_context(tc.tile_pool(name="wvp", bufs=1))
        ART = mybir.dt.float16  # AllReduce payload dtype (E fits fp16 range)
        E_sb = bigbuf.tile([P, LC * Q], ART)          # [128, 8192]
        wvT_sb = wvp.tile([P, DC, D], CDT)
        nc.sync.dma_start(out=wvT_sb, in_=wvT_v)      # overlaps C on DMA
        nc.sync.dma_start(out=bv_sb, in_=bv_in[:, :])
        AR_BOUNDS = []  # (start_col, n_cols) per AR chunk
        acc = 0
        for n_lc in AR_LCS:
            AR_BOUNDS.append((acc * Q, n_lc * Q))
            acc += n_lc
        E_drams = [dram.tile([P, w], ART, name=f"E_dram_{i}")
                   for i, (_, w) in enumerate(AR_BOUNDS)]
        denom_drams = [dram.tile([P, w], ART, addr_space="Shared",
                                 name=f"denom_dram_{i}")
                       for i, (_, w) in enumerate(AR_BOUNDS)]

        for sl in range(LC // SLAB):
            if sl < 2:
                kT_t = kT_pre[sl]
            else:
                kT_t = kslabs.tile([P, DC, SLAB * P], CDT, tag="kT")
                nc.sync.dma_start(
                    out=kT_t,
                    in_=kT_v[:, :, sl * SLAB * P:(sl + 1) * SLAB * P])
            for s in range(SLAB):
                lc = sl * SLAB + s
                ps_s = ps4.tile([P, Q], F32, tag="ps")
                for dc in range(DC):
                    nc.tensor.matmul(
                        ps_s,
                        kT_t[:, dc, s * P:(s + 1) * P],
                        qkT_sb[:, dc, :],
                        start=(dc == 0), stop=False,
                    )
                nc.tensor.matmul(
                    ps_s, ones_row[:, :P], qkb_sb,
                    start=False, stop=True,
                )
                nc.scalar.activation(
                    out=E_sb[:, lc * Q:(lc + 1) * Q], in_=ps_s,
                    func=mybir.ActivationFunctionType.Exp, scale=SCALE,
                )
            # one batched E store per slab (4 l-chunks, 2KB lines)
            g0 = sl * SLAB * Q
            ar_i = next(i for i, (s0, w) in enumerate(AR_BOUNDS)
                        if s0 <= g0 < s0 + w)
            s0, W_ar = AR_BOUNDS[ar_i]
            off = g0 - s0
            W = SLAB * Q
            nc.sync.dma_start(
                out=E_drams[ar_i][:, off:off + W],
                in_=E_sb[:, g0:g0 + W],
            )
            # kick off this chunk's AllReduce as soon as its last
            # E slice is stored (overlaps the rest of phase C)
            if off + W == W_ar:
                nc.gpsimd.collective_compute(
                    "AllReduce", mybir.AluOpType.add,
                    replica_groups=[list(range(N_CORES))],
                    ins=[E_drams[ar_i].opt()],
                    outs=[denom_drams[ar_i].opt()],
                )
        ps4_ctx.close()

        # ====== phases E+F interleaved per AR chunk: attn then av_T ======
        attnp = bigctx.enter_context(tc.tile_pool(name="attnp", bufs=1))
        rscr = bigctx.enter_context(tc.tile_pool(name="rscr", bufs=2))
        denom_sb = attnp.tile([P, LC * Q], ART)
        attn_sb = attnp.tile([P, LC * Q], CDT)
        CH = 512  # DVE/ACT chunk (2 l-chunks): short lead-in before each F group
        with (tc.tile_pool(name="accump", bufs=1, space="PSUM") as accump,
              tc.tile_pool(name="vslabs", bufs=6) as vslabs):
            av_ps = [accump.tile([P, Q], F32, name=f"av_ps_{dc}")
                     for dc in range(DC)]
            lc0 = 0
            for ar_i, (s0, W_ar) in enumerate(AR_BOUNDS):
                for j in range(W_ar // CH):
                    off = j * CH
                    sli = slice(s0 + off, s0 + off + CH)
                    nc.sync.dma_start(out=denom_sb[:, sli],
                                      in_=denom_drams[ar_i][:, off:off + CH])
                    r32 = rscr.tile([P, CH], F32, tag="r32")
                    nc.scalar.copy(r32, denom_sb[:, sli])  # fp16 -> fp32
                    nc.vector.reciprocal_approx_fast(r32, r32)
                    nc.vector.tensor_tensor(attn_sb[:, sli], E_sb[:, sli],
                                            r32, op=mybir.AluOpType.mult)
                n_lc = W_ar // Q
                for lc in range(lc0, lc0 + n_lc):
                    v_t = vslabs.tile([P, D], CDT, tag="vt")
                    nc.sync.dma_start(out=v_t, in_=v_v[lc, :, :])
                    at = attn_sb[:, lc * Q:(lc + 1) * Q]
                    for dc in range(DC):
                        nc.tensor.matmul(
                            av_ps[dc], v_t[:, dc * P:(dc + 1) * P], at,
                            start=(lc == 0), stop=(lc == LC - 1),
                        )
                lc0 += n_lc
            for dc in range(DC):
                nc.scalar.copy(avT_sb[:, dc, :], av_ps[dc])

        # ================ phase G: rowsum + out projection ===============
        with (tc.tile_pool(name="outp", bufs=2, space="PSUM") as outp,
              tc.tile_pool(name="rsp", bufs=1, space="PSUM") as rsp,
              tc.tile_pool(name="outsb", bufs=2) as outsb):
            rs_ps = rsp.tile([1, Q], F32)
            for lc in range(LC):
                nc.tensor.matmul(
                    rs_ps, ones_col, attn_sb[:, lc * Q:(lc + 1) * Q],
                    start=(lc == 0), stop=(lc == LC - 1),
                )
            nc.scalar.copy(rs_sb, rs_ps)

            # out[q,e] = sum_d av_T[d, q-chunk].T @ WvT[d, e] + rs * bv
            NB = D // 512
            for qm in range(Q // P):
                for eb in range(NB):
                    ps_o = outp.tile([P, 512], F32, tag="ps_out")
                    for dc in range(DC):
                        nc.tensor.matmul(
                            ps_o,
                            avT_sb[:, dc, qm * P:(qm + 1) * P],
                            wvT_sb[:, dc, eb * 512:(eb + 1) * 512],
                            start=(dc == 0), stop=False,
                        )
                    nc.tensor.matmul(
                        ps_o,
                        rs_sb[:, qm * P:(qm + 1) * P],
                        bv_sb[:, eb * 512:(eb + 1) * 512],
                        start=False, stop=True,
                    )
                    o_sb = outsb.tile([P, 512], F32, tag="o_sb")
                    nc.vector.tensor_copy(o_sb, ps_o)
                    nc.sync.dma_start(
                        out=out_v[:, qm, eb * 512:(eb + 1) * 512], in_=o_sb)
        bigctx.close()
        kslab_ctx.close()

    nc.compile()
    return nc


def _prep_inputs(q, k, v, Wq, bq, Wk, bk, Wv, bv):
    """Shard + pre-transpose + fold weights on host. in_maps for 8 cores."""
    cnp = _np_cdt()
    f32 = np.float32
    f64 = np.float64

    def c(x):
        return np.ascontiguousarray(np.asarray(x, dtype=f32), dtype=cnp)

    # host-folded q-side weights (f64 accumulation for precision)
    Wq64 = np.asarray(Wq, dtype=f64)
    Wk64 = np.asarray(Wk, dtype=f64)
    bq64 = np.asarray(bq, dtype=f64)
    bk64 = np.asarray(bk, dtype=f64)
    M = c(Wq64.T @ Wk64)                               # [D, D]
    m0 = c((bq64 @ Wk64).reshape(1, D))                # [1, D]
    wqkb = c((Wq64.T @ bk64).reshape(EC, P).T)         # [P, EC]
    s0 = c(np.array([[bq64 @ bk64]], dtype=f64))       # [1, 1]
    wvT = c(np.asarray(Wv, dtype=f32).T)
    bv_ = c(np.asarray(bv, dtype=f32).reshape(1, D))
    ones_r = np.ones((1, Q), dtype=cnp)
    ones_c = np.ones((P, 1), dtype=cnp)

    in_maps = []
    for b in range(B):
        in_maps.append({
            "kT": c(np.asarray(k[b], dtype=f32).T),
            "v_in": c(v[b]),
            "qT": c(np.asarray(q[b, :Q], dtype=f32).T),
            "m_in": M,
            "wvT": wvT,
            "m0_in": m0,
            "wqkb_in": wqkb,
            "s0_in": s0,
            "bv_in": bv_,
            "ones_r": ones_r,
            "ones_c": ones_c,
        })
    return in_maps


def kernel(q, k, v, Wq, bq, Wk, bk, Wv, bv, _trace=False):
    q = np.asarray(q)
    k = np.asarray(k)
    v = np.asarray(v)
    if "nc" not in _cached:
        _cached["nc"] = build_kernel()
    nc = _cached["nc"]
    in_maps = _prep_inputs(q, k, v, Wq, bq, Wk, bk, Wv, bv)
    res = bass_utils.run_bass_kernel_spmd(
        nc, in_maps, core_ids=list(range(N_CORES)), trace=_trace)
    out = np.stack([res.results[c]["out"] for c in range(N_CORES)], axis=0)
    if _trace:
        _cached["last_results"] = res
    return out.astype(np.float32)


if __name__ == "__main__":
    rng = np.random.default_rng(0)
    ins = {
        "q": rng.standard_normal((B, L, D)).astype(np.float32),
        "k": rng.standard_normal((B, L, D)).astype(np.float32),
        "v": rng.standard_normal((B, L, D)).astype(np.float32),
        "Wq": (rng.standard_normal((D, D)) * 0.02).astype(np.float32),
        "bq": (rng.standard_normal(D) * 0.02).astype(np.float32),
        "Wk": (rng.standard_normal((D, D)) * 0.02).astype(np.float32),
        "bk": (rng.standard_normal(D) * 0.02).astype(np.float32),
        "Wv": (rng.standard_normal((D, D)) * 0.02).astype(np.float32),
        "bv": (rng.standard_normal(D) * 0.02).astype(np.float32),
    }
    out = kernel(**ins)
    print("out", out.shape, out.dtype)
